# revision 21
# baseline (speedup 1.0000x reference)
"""ROIAlign Trainium2 kernel.

Problem: feat [2,256,64,64] f32, rois [1000,5] f32 -> out [1000,256,7,7] f32
(spatial_scale=1/16, out 7x7, sampling_ratio=2, aligned=True).

Strategy (sharded across ROIs, 125 per core on 8 cores):
  - ROI boxes are small (<=7.5 feature px): each ROI's 14x14 bilinear
    sample grid reads a fixed 9x9 window of feature cells.
  - Host precomputes per ROI: window origin and a combined
    bilinear+avgpool weight matrix W [81, 49].
  - Feature map is host-transposed to featT [(n,y,x), C] so one window
    y-row (9 cells x 256 ch) is one contiguous 9KB run.
  - Device per 14-ROI batch: one indirect DMA gathers 126 rows (one per
    (roi, dy), contiguous 2304 elems each) -> gt [126, 2304].
  - Per ROI: SBUF->SBUF DMA rearranges its [9, 9*256] slice into
    regionT [81, 256]; matmul W[81,49]^T @ regionT[81,256] (float32r)
    -> PSUM; two ROIs share one PSUM bank [49, 512] (cols 0:256 / 256:512);
    one DVE copy moves the pair to SBUF; per-batch DMA writes [49, 7*512].
  - Host reassembles [49, 63*512] per core into [K, 256, 7, 7].
"""

import numpy as np

# ---- problem constants (hardcoded per contract) ----
N, C, H, W = 2, 256, 64, 64
K = 1000
N_CORES = 8
KPC = K // N_CORES  # 125 rois per core
SPATIAL_SCALE = 0.0625
OUT_H = OUT_W = 7
SR = 2
NBINS = OUT_H * OUT_W  # 49
SY = OUT_H * SR  # 14
WIN = 9  # window edge (cells); 9 always covers max span
RCELLS = WIN * WIN  # 81
GB = 14  # rois per indirect-DMA gather batch (126 partitions)
NBATCH = (KPC + GB - 1) // GB  # 9 batches: 8x14 + 1x13
NPAIRS = (KPC + 1) // 2  # 63 pair slots (last is a single)
PPB = (GB + 1) // 2  # pair slots per batch: 7


def _f32(x):
    return np.float32(x)


def _interp_axis(c, size):
    """Mirror reference._interp_axis in float32 numpy."""
    valid = (c >= _f32(-1.0)) & (c <= _f32(size))
    cc = np.maximum(c, _f32(0.0))
    low = np.floor(cc).astype(np.int32)
    at_end = low >= size - 1
    low = np.where(at_end, size - 1, low).astype(np.int32)
    high = np.where(at_end, size - 1, low + 1).astype(np.int32)
    cc = np.where(at_end, low.astype(np.float32), cc)
    frac = (cc - low.astype(np.float32)).astype(np.float32)
    return low, high, frac, valid


def _build_tables(rois):
    """Per-ROI window row-start indices [K, 9] and weights [K, 81, 49]."""
    rois = np.asarray(rois, dtype=np.float32)
    k = rois.shape[0]
    bidx = rois[:, 0].astype(np.int32)
    off = _f32(0.5)
    scale = _f32(SPATIAL_SCALE)
    x1 = rois[:, 1] * scale - off
    y1 = rois[:, 2] * scale - off
    x2 = rois[:, 3] * scale - off
    y2 = rois[:, 4] * scale - off
    rw = x2 - x1
    rh = y2 - y1
    bin_h = rh / _f32(OUT_H)
    bin_w = rw / _f32(OUT_W)
    s = np.arange(SY, dtype=np.float32) + _f32(0.5)
    ys = y1[:, None] + s[None, :] * (bin_h[:, None] / _f32(SR))  # [k, 14]
    xs = x1[:, None] + s[None, :] * (bin_w[:, None] / _f32(SR))  # [k, 14]

    yl, yh, ly, vy = _interp_axis(ys, H)
    xl, xh, lx, vx = _interp_axis(xs, W)
    assert vy.all() and vx.all(), "unexpected out-of-range sample (mask unhandled)"
    hy, hx = _f32(1.0) - ly, _f32(1.0) - lx

    y0 = np.minimum(yl.min(axis=1), H - WIN).astype(np.int32)
    x0 = np.minimum(xl.min(axis=1), W - WIN).astype(np.int32)
    assert (yh.max(axis=1) - y0 < WIN).all() and (xh.max(axis=1) - x0 < WIN).all()

    # per-axis window weights Wy/Wx [k, WIN, 7]
    wy = np.zeros((k, WIN, OUT_H), dtype=np.float32)
    wx = np.zeros((k, WIN, OUT_W), dtype=np.float32)
    ar = np.arange(k)
    for sidx in range(SY):
        ph = sidx // SR
        np.add.at(wy, (ar, yl[:, sidx] - y0, ph), hy[:, sidx])
        np.add.at(wy, (ar, yh[:, sidx] - y0, ph), ly[:, sidx])
        np.add.at(wx, (ar, xl[:, sidx] - x0, ph), hx[:, sidx])
        np.add.at(wx, (ar, xh[:, sidx] - x0, ph), lx[:, sidx])

    # combined [k, 81, 49]; fold the 1/(sr*sr) pool mean into the weights
    wmat = (
        wy[:, :, None, :, None] * wx[:, None, :, None, :] * _f32(1.0 / (SR * SR))
    ).reshape(k, RCELLS, NBINS)

    dy = np.arange(WIN, dtype=np.int32)
    rowstart = (
        bidx[:, None] * (H * W) + (y0[:, None] + dy[None, :]) * W + x0[:, None]
    )  # [k, 9]
    return rowstart.astype(np.int32), wmat.astype(np.float32)


def _build_program():
    import concourse.bacc as bacc
    import concourse.bass as bass
    import concourse.mybir as mybir
    from concourse.tile import TileContext

    f32r = mybir.dt.float32r
    nc = bacc.Bacc("TRN2", target_bir_lowering=False, debug=False)
    featT = nc.dram_tensor("featT", [N * H * W, C], f32r, kind="ExternalInput")
    wtab = nc.dram_tensor("wtab", [RCELLS, KPC * NBINS], f32r, kind="ExternalInput")
    itab = nc.dram_tensor("itab", [GB * WIN, NBATCH], mybir.dt.int32, kind="ExternalInput")
    outp = nc.dram_tensor("outp", [NBINS, NPAIRS * 2 * C], mybir.dt.float32, kind="ExternalOutput")

    with TileContext(nc) as tc:
        with (
            tc.tile_pool(name="const", bufs=1) as const_pool,
            tc.tile_pool(name="gather", bufs=6) as gather_pool,
            tc.tile_pool(name="region", bufs=16) as region_pool,
            tc.tile_pool(name="outbuf", bufs=3) as out_pool,
            tc.tile_pool(name="psum", bufs=4, space="PSUM") as psum_pool,
        ):
            w_sb = const_pool.tile([RCELLS, KPC * NBINS], f32r)
            i_sb = const_pool.tile([GB * WIN, NBATCH], mybir.dt.int32)
            nc.sync.dma_start(out=w_sb[:], in_=wtab[:])
            nc.scalar.dma_start(out=i_sb[:], in_=itab[:])

            eng = [nc.sync, nc.scalar]  # variant C: scalar ring back, no memset
            ecnt = 0
            for b in range(NBATCH):
                rc = min(GB, KPC - b * GB)  # rois in this batch (14 or 13)
                gt = gather_pool.tile([GB * WIN, WIN * C], f32r)
                nc.gpsimd.indirect_dma_start(
                    out=gt[: rc * WIN, :],
                    out_offset=None,
                    in_=featT[:],
                    in_offset=bass.IndirectOffsetOnAxis(
                        ap=i_sb[: rc * WIN, b : b + 1], axis=0
                    ),
                )
                out_sb = out_pool.tile([NBINS, PPB * 2 * C], mybir.dt.float32)

                for r0 in range(0, rc, 4):
                    npair = min(4, rc - r0)
                    ps = psum_pool.tile([NBINS, 4 * C], mybir.dt.float32)
                    for j in range(npair):
                        r = r0 + j
                        kk = b * GB + r
                        region = region_pool.tile([RCELLS, C], f32r)
                        src = gt[r * WIN : (r + 1) * WIN, :].rearrange(
                            "p (a c) -> p a c", c=C
                        )
                        eng[ecnt % 2].dma_start(out=region[:], in_=src)
                        ecnt += 1
                        nc.tensor.matmul(
                            out=ps[:, j * C : (j + 1) * C],
                            lhsT=w_sb[:, kk * NBINS : (kk + 1) * NBINS],
                            rhs=region[:],
                            start=True,
                            stop=True,
                        )
                    nc.vector.tensor_copy(
                        out=out_sb[:, r0 * C : (r0 + npair) * C],
                        in_=ps[:, : npair * C],
                    )
                eng[b % 2].dma_start(
                    out=outp[:, b * PPB * 2 * C : b * PPB * 2 * C + rc * C],
                    in_=out_sb[:, : rc * C],
                )
    nc.finalize()
    return nc


def _prepare_inputs(feat, rois):
    feat = np.asarray(feat, dtype=np.float32)
    rois = np.asarray(rois, dtype=np.float32)
    featT = np.ascontiguousarray(feat.transpose(0, 2, 3, 1).reshape(N * H * W, C))
    rowstart, wmat = _build_tables(rois)  # [K,9] i32, [K,81,49] f32
    in_maps = []
    for c in range(N_CORES):
        sl = slice(c * KPC, (c + 1) * KPC)
        rs = rowstart[sl]  # [125, 9]
        itab = np.zeros((GB * WIN, NBATCH), dtype=np.int32)
        for b in range(NBATCH):
            rc = min(GB, KPC - b * GB)
            blk = rs[b * GB : b * GB + rc].reshape(rc * WIN)
            itab[: rc * WIN, b] = blk
        wtab = np.ascontiguousarray(
            wmat[sl].transpose(1, 0, 2).reshape(RCELLS, KPC * NBINS)
        )  # [81, 125*49]
        in_maps.append({"featT": featT, "wtab": wtab, "itab": itab})
    return in_maps


def _assemble_core(dev):
    """[49, 63*512] device buffer -> [125, 256, 7, 7]."""
    dev = dev.reshape(NBINS, NPAIRS * 2, C)
    out = np.empty((KPC, C, OUT_H, OUT_W), dtype=np.float32)
    for kk in range(KPC):
        b, r = divmod(kk, GB)
        col = b * 2 * PPB + r
        out[kk] = dev[:, col].T.reshape(C, OUT_H, OUT_W)
    return out


def _assemble(results):
    return np.concatenate(
        [_assemble_core(results[c]["outp"]) for c in range(N_CORES)], axis=0
    )


def _run(feat, rois, trace=False):
    from concourse.bass_utils import run_bass_kernel_spmd

    nc = _build_program()
    in_maps = _prepare_inputs(feat, rois)
    res = run_bass_kernel_spmd(nc, in_maps, core_ids=list(range(N_CORES)), trace=trace)
    return _assemble(res.results), res


def kernel(feat, rois):
    out, _ = _run(feat, rois, trace=False)
    return out


# revision 22
# speedup vs baseline: 1.2075x; 1.2075x over previous
"""ROIAlign Trainium2 kernel.

Problem: feat [2,256,64,64] f32, rois [1000,5] f32 -> out [1000,256,7,7] f32
(spatial_scale=1/16, out 7x7, sampling_ratio=2, aligned=True).

Strategy (sharded across ROIs, 125 per core on 8 cores):
  - ROI boxes are small (<=7.5 feature px): each ROI's 14x14 bilinear
    sample grid reads a fixed 9x9 window of feature cells.
  - Host precomputes per ROI: window origin and a combined
    bilinear+avgpool weight matrix W [81, 49].
  - Feature map is host-transposed to featT [(n,y,x), C] so one window
    y-row (9 cells x 256 ch) is one contiguous 9KB run.
  - Device per 14-ROI batch: one indirect DMA gathers 126 rows (one per
    (roi, dy), contiguous 2304 elems each) -> gt [126, 2304].
  - Per ROI: SBUF->SBUF DMA rearranges its [9, 9*256] slice into
    regionT [81, 256]; matmul W[81,49]^T @ regionT[81,256] (float32r)
    -> PSUM; two ROIs share one PSUM bank [49, 512] (cols 0:256 / 256:512);
    one DVE copy moves the pair to SBUF; per-batch DMA writes [49, 7*512].
  - Host reassembles [49, 63*512] per core into [K, 256, 7, 7].
"""

import numpy as np

# ---- problem constants (hardcoded per contract) ----
N, C, H, W = 2, 256, 64, 64
K = 1000
N_CORES = 8
KPC = K // N_CORES  # 125 rois per core
SPATIAL_SCALE = 0.0625
OUT_H = OUT_W = 7
SR = 2
NBINS = OUT_H * OUT_W  # 49
SY = OUT_H * SR  # 14
WIN = 9  # window edge (cells); 9 always covers max span
RCELLS = WIN * WIN  # 81
GB = 14  # rois per indirect-DMA gather batch (126 partitions)
NBATCH = (KPC + GB - 1) // GB  # 9 batches: 8x14 + 1x13
NPAIRS = (KPC + 1) // 2  # 63 pair slots (last is a single)
PPB = (GB + 1) // 2  # pair slots per batch: 7


def _f32(x):
    return np.float32(x)


def _interp_axis(c, size):
    """Mirror reference._interp_axis in float32 numpy."""
    valid = (c >= _f32(-1.0)) & (c <= _f32(size))
    cc = np.maximum(c, _f32(0.0))
    low = np.floor(cc).astype(np.int32)
    at_end = low >= size - 1
    low = np.where(at_end, size - 1, low).astype(np.int32)
    high = np.where(at_end, size - 1, low + 1).astype(np.int32)
    cc = np.where(at_end, low.astype(np.float32), cc)
    frac = (cc - low.astype(np.float32)).astype(np.float32)
    return low, high, frac, valid


def _build_tables(rois):
    """Per-ROI window row-start indices [K, 9] and weights [K, 81, 49]."""
    rois = np.asarray(rois, dtype=np.float32)
    k = rois.shape[0]
    bidx = rois[:, 0].astype(np.int32)
    off = _f32(0.5)
    scale = _f32(SPATIAL_SCALE)
    x1 = rois[:, 1] * scale - off
    y1 = rois[:, 2] * scale - off
    x2 = rois[:, 3] * scale - off
    y2 = rois[:, 4] * scale - off
    rw = x2 - x1
    rh = y2 - y1
    bin_h = rh / _f32(OUT_H)
    bin_w = rw / _f32(OUT_W)
    s = np.arange(SY, dtype=np.float32) + _f32(0.5)
    ys = y1[:, None] + s[None, :] * (bin_h[:, None] / _f32(SR))  # [k, 14]
    xs = x1[:, None] + s[None, :] * (bin_w[:, None] / _f32(SR))  # [k, 14]

    yl, yh, ly, vy = _interp_axis(ys, H)
    xl, xh, lx, vx = _interp_axis(xs, W)
    assert vy.all() and vx.all(), "unexpected out-of-range sample (mask unhandled)"
    hy, hx = _f32(1.0) - ly, _f32(1.0) - lx

    y0 = np.minimum(yl.min(axis=1), H - WIN).astype(np.int32)
    x0 = np.minimum(xl.min(axis=1), W - WIN).astype(np.int32)
    assert (yh.max(axis=1) - y0 < WIN).all() and (xh.max(axis=1) - x0 < WIN).all()

    # per-axis window weights Wy/Wx [k, WIN, 7]
    wy = np.zeros((k, WIN, OUT_H), dtype=np.float32)
    wx = np.zeros((k, WIN, OUT_W), dtype=np.float32)
    ar = np.arange(k)
    for sidx in range(SY):
        ph = sidx // SR
        np.add.at(wy, (ar, yl[:, sidx] - y0, ph), hy[:, sidx])
        np.add.at(wy, (ar, yh[:, sidx] - y0, ph), ly[:, sidx])
        np.add.at(wx, (ar, xl[:, sidx] - x0, ph), hx[:, sidx])
        np.add.at(wx, (ar, xh[:, sidx] - x0, ph), lx[:, sidx])

    # combined [k, 81, 49]; fold the 1/(sr*sr) pool mean into the weights
    wmat = (
        wy[:, :, None, :, None] * wx[:, None, :, None, :] * _f32(1.0 / (SR * SR))
    ).reshape(k, RCELLS, NBINS)

    dy = np.arange(WIN, dtype=np.int32)
    rowstart = (
        bidx[:, None] * (H * W) + (y0[:, None] + dy[None, :]) * W + x0[:, None]
    )  # [k, 9]
    return rowstart.astype(np.int32), wmat.astype(np.float32)


def _build_program():
    import concourse.bacc as bacc
    import concourse.bass as bass
    import concourse.mybir as mybir
    from concourse.tile import TileContext

    f32r = mybir.dt.float32r
    nc = bacc.Bacc("TRN2", target_bir_lowering=False, debug=False)
    featT = nc.dram_tensor("featT", [N * H * W, C], f32r, kind="ExternalInput")
    wtab = nc.dram_tensor("wtab", [RCELLS, KPC * NBINS], f32r, kind="ExternalInput")
    itab = nc.dram_tensor("itab", [GB * WIN, NBATCH], mybir.dt.int32, kind="ExternalInput")
    outp = nc.dram_tensor("outp", [NBINS, NPAIRS * 2 * C], mybir.dt.float32, kind="ExternalOutput")

    with TileContext(nc) as tc:
        with (
            tc.tile_pool(name="const", bufs=1) as const_pool,
            tc.tile_pool(name="gather", bufs=6) as gather_pool,
            tc.tile_pool(name="region", bufs=16) as region_pool,
            tc.tile_pool(name="outbuf", bufs=3) as out_pool,
            tc.tile_pool(name="psum", bufs=4, space="PSUM") as psum_pool,
        ):
            w_sb = const_pool.tile([RCELLS, KPC * NBINS], f32r)
            i_sb = const_pool.tile([GB * WIN, NBATCH], mybir.dt.int32)
            nc.sync.dma_start(out=w_sb[:], in_=wtab[:])
            nc.scalar.dma_start(out=i_sb[:], in_=itab[:])

            # rearranges round-robin over all three DGE paths; gpsimd also
            # carries the batch gathers so it gets a smaller share
            eng3 = [nc.sync, nc.scalar, nc.gpsimd, nc.sync, nc.scalar, nc.gpsimd, nc.sync]
            eng = [nc.sync, nc.scalar]
            ecnt = 0
            for b in range(NBATCH):
                rc = min(GB, KPC - b * GB)  # rois in this batch (14 or 13)
                gt = gather_pool.tile([GB * WIN, WIN * C], f32r)
                nc.gpsimd.indirect_dma_start(
                    out=gt[: rc * WIN, :],
                    out_offset=None,
                    in_=featT[:],
                    in_offset=bass.IndirectOffsetOnAxis(
                        ap=i_sb[: rc * WIN, b : b + 1], axis=0
                    ),
                )
                out_sb = out_pool.tile([NBINS, PPB * 2 * C], mybir.dt.float32)

                for r0 in range(0, rc, 4):
                    npair = min(4, rc - r0)
                    ps = psum_pool.tile([NBINS, 4 * C], mybir.dt.float32)
                    for j in range(npair):
                        r = r0 + j
                        kk = b * GB + r
                        region = region_pool.tile([RCELLS, C], f32r)
                        src = gt[r * WIN : (r + 1) * WIN, :].rearrange(
                            "p (a c) -> p a c", c=C
                        )
                        eng3[ecnt % 7].dma_start(out=region[:], in_=src)
                        ecnt += 1
                        nc.tensor.matmul(
                            out=ps[:, j * C : (j + 1) * C],
                            lhsT=w_sb[:, kk * NBINS : (kk + 1) * NBINS],
                            rhs=region[:],
                            start=True,
                            stop=True,
                        )
                    nc.vector.tensor_copy(
                        out=out_sb[:, r0 * C : (r0 + npair) * C],
                        in_=ps[:, : npair * C],
                    )
                eng[b % 2].dma_start(
                    out=outp[:, b * PPB * 2 * C : b * PPB * 2 * C + rc * C],
                    in_=out_sb[:, : rc * C],
                )
    nc.finalize()
    return nc


def _prepare_inputs(feat, rois):
    feat = np.asarray(feat, dtype=np.float32)
    rois = np.asarray(rois, dtype=np.float32)
    featT = np.ascontiguousarray(feat.transpose(0, 2, 3, 1).reshape(N * H * W, C))
    rowstart, wmat = _build_tables(rois)  # [K,9] i32, [K,81,49] f32
    in_maps = []
    for c in range(N_CORES):
        sl = slice(c * KPC, (c + 1) * KPC)
        rs = rowstart[sl]  # [125, 9]
        itab = np.zeros((GB * WIN, NBATCH), dtype=np.int32)
        for b in range(NBATCH):
            rc = min(GB, KPC - b * GB)
            blk = rs[b * GB : b * GB + rc].reshape(rc * WIN)
            itab[: rc * WIN, b] = blk
        wtab = np.ascontiguousarray(
            wmat[sl].transpose(1, 0, 2).reshape(RCELLS, KPC * NBINS)
        )  # [81, 125*49]
        in_maps.append({"featT": featT, "wtab": wtab, "itab": itab})
    return in_maps


def _assemble_core(dev):
    """[49, 63*512] device buffer -> [125, 256, 7, 7]."""
    dev = dev.reshape(NBINS, NPAIRS * 2, C)
    out = np.empty((KPC, C, OUT_H, OUT_W), dtype=np.float32)
    for kk in range(KPC):
        b, r = divmod(kk, GB)
        col = b * 2 * PPB + r
        out[kk] = dev[:, col].T.reshape(C, OUT_H, OUT_W)
    return out


def _assemble(results):
    return np.concatenate(
        [_assemble_core(results[c]["outp"]) for c in range(N_CORES)], axis=0
    )


def _run(feat, rois, trace=False):
    from concourse.bass_utils import run_bass_kernel_spmd

    nc = _build_program()
    in_maps = _prepare_inputs(feat, rois)
    res = run_bass_kernel_spmd(nc, in_maps, core_ids=list(range(N_CORES)), trace=trace)
    return _assemble(res.results), res


def kernel(feat, rois):
    out, _ = _run(feat, rois, trace=False)
    return out
